# revision 8
# baseline (speedup 1.0000x reference)
"""TRN2 Bass kernel for GPT-style causal self-attention with RoPE (bf16).

Reference (B=2, S=2048, D=1024, H=16, dk=64):
  qkv = hidden @ c_attn_w + c_attn_b; rope(q), rope(k) via position_ids;
  out = softmax(causal(q k^T / 8)) v, merged heads, @ c_proj_w + c_proj_b.

Sharding across 8 NeuronCores: core c = 4*b + g handles batch b and head
group g (4 heads = 256 dims). Each core computes its full S x S attention
for its heads and a row-sliced c_proj partial; the host sums the 4
partials per batch.

v2 schedule: the Scalar engine's softmax exp (~76us busy) and the PE
(~115us busy) are the two near-critical engines; v1 started exp at
~55us. v2 computes head-pair 0's q/k first, ropes them in halves, and
starts the scores+exp pipeline at ~12us, interleaving the remaining
QKV (v, head-pair 1) and V transposes into the PE stream behind the
score tiles. Chunk order: hp0 ascending (0..3), hp1 descending (3..0)
so the tail ends on the smallest chunk. PV of the previous chunk is
emitted at the midpoint of the current chunk's scores; proj(c) fires
once both head-pairs of chunk c are normalized. One shared 3-buf PSUM
pool serves QKV fills, PV accumulators and proj so all phases coexist
with the 2x2-bank score pool and the transpose pool (8 banks total).

wqkv DRAM layout (host-side) is [q01|k01|q23|k23|v0123] so head-pair 0
needs one contiguous 256-col load per k-chunk. Input DMA is spread
over 5 engine queues (hT striped over sync/vector/tensor by k-chunk,
first column-half first; weights on scalar; trig/consts on gpsimd).

Output per core: outT [1024, 2048] bf16 partial; host sums per batch.
"""

from contextlib import ExitStack

import numpy as np
import ml_dtypes

import concourse.bacc as bacc
import concourse.tile as tile
import concourse.mybir as mybir
from concourse.bass_utils import run_bass_kernel_spmd

f32 = mybir.dt.float32
bf16 = mybir.dt.bfloat16
AF = mybir.ActivationFunctionType
ALU = mybir.AluOpType

S = 2048
D = 1024
HD = 256           # head dims per core (4 heads x 64)
SB = S // 128      # 16
KC = D // 128      # 8
NCH = S // 512     # 4
BF = ml_dtypes.bfloat16


def build_attention_nc(with_bias=False, num_devices=8):
    nc = bacc.Bacc("TRN2", target_bir_lowering=False, debug=False,
                   num_devices=num_devices)

    hT_d = nc.dram_tensor("hT", [D, S], bf16, kind="ExternalInput")
    wqkv_d = nc.dram_tensor("wqkv", [D, 768], bf16, kind="ExternalInput")
    cosT_d = nc.dram_tensor("cosT", [128, S], bf16, kind="ExternalInput")
    sinT_d = nc.dram_tensor("sinT", [128, S], bf16, kind="ExternalInput")
    wp_d = nc.dram_tensor("wp", [HD, D], bf16, kind="ExternalInput")
    bp_d = nc.dram_tensor("bp", [128, 8], f32, kind="ExternalInput")
    mask01_d = nc.dram_tensor("mask01", [128, 128], bf16, kind="ExternalInput")
    ones64_d = nc.dram_tensor("ones64", [128, 64], bf16, kind="ExternalInput")
    ident_d = nc.dram_tensor("ident", [128, 128], bf16, kind="ExternalInput")
    if with_bias:
        bqkv_d = nc.dram_tensor("bqkv", [1, 768], bf16, kind="ExternalInput")
        onesrow_d = nc.dram_tensor("ones_row", [1, 512], bf16,
                                   kind="ExternalInput")
    outT_d = nc.dram_tensor("outT", [D, S], bf16, kind="ExternalOutput")

    with tile.TileContext(nc) as tc, ExitStack() as top:
        const = top.enter_context(tc.tile_pool(name="const", bufs=1))
        ident = const.tile([128, 128], bf16, tag="ident")
        mask2 = const.tile([128, 2, 128], bf16, tag="mask2")
        bp_sb = const.tile([128, 8], f32, tag="bp")
        ones64 = const.tile([128, 64], bf16, tag="ones64")
        if with_bias:
            bqkv_sb = const.tile([1, 768], bf16, tag="bqkv")
            nc.sync.dma_start(bqkv_sb[:], bqkv_d.ap())
            ones_row = const.tile([1, 512], bf16, tag="ones_row")
            nc.sync.dma_start(ones_row[:], onesrow_d.ap())

        persist = top.enter_context(tc.tile_pool(name="persist", bufs=1))
        qT = [persist.tile([128, S], bf16, tag=f"qT{hp}", name=f"qT{hp}")
              for hp in range(2)]
        kT = [persist.tile([128, S], bf16, tag=f"kT{hp}", name=f"kT{hp}")
              for hp in range(2)]
        v_sb = persist.tile([128, SB, 4, 65], bf16, tag="v")
        wp_sb = persist.tile([128, 2, D], bf16, tag="wp")
        attnT = [persist.tile([128, S], bf16, tag=f"attnT{hp}",
                              name=f"attnT{hp}") for hp in range(2)]
        cosT = persist.tile([128, S], bf16, tag="cosT")
        sinT = persist.tile([128, S], bf16, tag="sinT")

        # ---------------- input DMA, spread over 5 queues ----------------
        # gpsimd: small consts + trig (needed by rope at ~7us)
        nc.gpsimd.dma_start(mask2[:, 0, :], mask01_d.ap())
        nc.gpsimd.dma_start(mask2[:, 1, :], mask01_d.ap())
        nc.gpsimd.dma_start(cosT[:], cosT_d.ap())
        nc.gpsimd.dma_start(sinT[:], sinT_d.ap())
        nc.gpsimd.dma_start(ident[:], ident_d.ap())
        nc.gpsimd.dma_start(ones64[:], ones64_d.ap())

        # scalar: hp0 weights first (wqkv cols = [q01|k01|q23|k23|v]),
        # then odd hT chunks; the queue is free again before exp starts.
        w_pool = top.enter_context(tc.tile_pool(name="w", bufs=1))
        w_sb = [w_pool.tile([128, 768], bf16, tag=f"w{kc}", name=f"w{kc}")
                for kc in range(KC)]
        for kc in range(KC):
            nc.scalar.dma_start(w_sb[kc][:, 0:256],
                                wqkv_d.ap()[kc * 128:(kc + 1) * 128, 0:256])
        # hT: even chunks on sync, odd on scalar, col-half 0 first.
        hT_pool = top.enter_context(tc.tile_pool(name="hT", bufs=1))
        hT_sb = [hT_pool.tile([128, S], bf16, tag=f"hT{kc}", name=f"hT{kc}")
                 for kc in range(KC)]
        for half in range(2):
            csl = slice(half * 1024, (half + 1) * 1024)
            for kc in range(KC):
                eng = nc.sync if kc % 2 == 0 else nc.scalar
                eng.dma_start(hT_sb[kc][:, csl],
                              hT_d.ap()[kc * 128:(kc + 1) * 128, csl])
        # later-needed weights on sync, behind hT
        for kc in range(KC):
            nc.sync.dma_start(w_sb[kc][:, 512:768],
                              wqkv_d.ap()[kc * 128:(kc + 1) * 128, 512:768])
        for kc in range(KC):
            nc.sync.dma_start(w_sb[kc][:, 256:512],
                              wqkv_d.ap()[kc * 128:(kc + 1) * 128, 256:512])
        for kc2 in range(2):
            nc.sync.dma_start(wp_sb[:, kc2, :],
                              wp_d.ap()[kc2 * 128:(kc2 + 1) * 128, :])
        nc.gpsimd.dma_start(bp_sb[:], bp_d.ap())

        # ---------------- pools ----------------
        acc_ps = top.enter_context(
            tc.tile_pool(name="acc_ps", bufs=3, space="PSUM"))
        tr_ps = top.enter_context(
            tc.tile_pool(name="tr_ps", bufs=1, space="PSUM"))
        _lazy = {}

        def st_ps_tile():
            if "st" not in _lazy:
                _lazy["st"] = top.enter_context(
                    tc.tile_pool(name="st_ps", bufs=2, space="PSUM",
                                 side="right"))
                _lazy["n"] = 0
            _lazy["n"] += 1
            return _lazy["st"].tile([128, 2, 512], f32, tag="st_p",
                                    name=f"st_p{_lazy['n']}")

        pt_pool = top.enter_context(tc.tile_pool(name="pt", bufs=30))
        u_pool = top.enter_context(tc.tile_pool(name="u", bufs=3))
        nrm_pool = top.enter_context(tc.tile_pool(name="nrm", bufs=2))
        pj_sb = top.enter_context(tc.tile_pool(name="pj_sb", bufs=3))
        vT_pool = top.enter_context(tc.tile_pool(name="vT", bufs=1))
        rope_pool = top.enter_context(tc.tile_pool(name="rope", bufs=4))
        qsw_pool = top.enter_context(tc.tile_pool(name="qsw", bufs=2))
        qcos_pool = top.enter_context(tc.tile_pool(name="qcos", bufs=1))

        vT_sb = [vT_pool.tile([128, S], bf16, tag=f"vT{t}", name=f"vT{t}")
                 for t in range(2)]
        nc.scalar.copy(v_sb[:, :, :, 64],
                       ones64[:].rearrange("p (a b) -> p a b", a=SB))

        # ---------------- building blocks ----------------
        # w col blocks: 0=q01 1=k01 2=q23 3=k23 4,5=v
        def fill_pair(blk, spair, qraw, inter=None):
            sls = [slice(sb_ * 512, (sb_ + 1) * 512) for sb_ in spair]
            ps = [acc_ps.tile([128, 512], f32, tag="acc", name=f"qkv_p{j}")
                  for j, _ in enumerate(spair)]
            for kc in range(KC):
                for p, sl in zip(ps, sls):
                    nc.tensor.matmul(
                        p[:], w_sb[kc][:, blk * 128:(blk + 1) * 128],
                        hT_sb[kc][:, sl], start=(kc == 0),
                        stop=(kc == KC - 1 and not with_bias))
            if with_bias:
                for p in ps:
                    nc.tensor.matmul(
                        p[:], bqkv_sb[:, blk * 128:(blk + 1) * 128],
                        ones_row[:], start=False, stop=True)
            if inter is not None:
                inter()
            for p, sl in zip(ps, sls):
                if blk >= 4:
                    nc.vector.tensor_copy(vT_sb[blk - 4][:, sl], p[:])
                else:
                    nc.vector.tensor_copy(qraw[:, sl], p[:])

        # rope for one w-block over a 1024-col half.
        def rope_half(blk, qraw, half):
            dest = (qT if blk % 2 == 0 else kT)[blk // 2]
            cols = slice(half * 1024, (half + 1) * 1024)
            qsw = qsw_pool.tile([128, 1024], bf16, tag="qsw")
            for b4 in range(4):
                sp = (b4 * 32 + 32) % 64 + 64 * (b4 // 2)
                nc.gpsimd.dma_start(qsw[b4 * 32:b4 * 32 + 32, :],
                                    qraw[sp:sp + 32, cols])
            qcos = qcos_pool.tile([128, 1024], bf16, tag="qcos")
            nc.vector.tensor_tensor(qcos[:], qraw[:, cols], cosT[:, cols],
                                    op=ALU.mult)
            nc.vector.tensor_tensor(qsw[:], qsw[:], sinT[:, cols],
                                    op=ALU.mult)
            nc.vector.tensor_tensor(dest[:, cols], qcos[:], qsw[:],
                                    op=ALU.add)

        _tr = iter([(t, sb_) for t in range(2) for sb_ in range(SB)])

        def transposes(n):
            for _ in range(n):
                nxt = next(_tr, None)
                if nxt is None:
                    return
                t, sb_ = nxt
                tp = tr_ps.tile([128, 128], bf16, tag="tp")
                nc.tensor.matmul(
                    tp[:], vT_sb[t][:, sb_ * 128:(sb_ + 1) * 128],
                    ident[:], is_transpose=True, start=True, stop=True)
                nc.vector.tensor_copy(
                    v_sb[:, sb_, 2 * t:2 * t + 2, 0:64],
                    tp[:].rearrange("p (h d) -> p h d", h=2))

        pts_map = {}

        def scores_tile(c, hp, kb):
            q0 = max(512 * c, 128 * kb)
            off = q0 - 512 * c
            st_p = st_ps_tile()
            for h2 in range(2):
                nc.tensor.matmul(
                    st_p[:, h2, off:512],
                    kT[hp][h2 * 64:(h2 + 1) * 64, kb * 128:(kb + 1) * 128],
                    qT[hp][h2 * 64:(h2 + 1) * 64, q0:512 * (c + 1)],
                    start=True, stop=True, tile_position=(h2 * 64, 0))
            pt = pt_pool.tile([128, 2, 512], bf16, tag="pt")
            nc.scalar.activation(pt[:, :, off:512], st_p[:, :, off:512],
                                 AF.Exp, scale=0.125)
            if 128 * kb >= 512 * c:
                nc.vector.tensor_tensor(pt[:, :, off:off + 128],
                                        pt[:, :, off:off + 128],
                                        mask2[:], op=ALU.mult)
            pts_map.setdefault((c, hp), []).append((kb, off, pt))

        def pv_pair(c, hp, den_eng=None):
            nkb = 4 * c + 4
            pts = pts_map.pop((c, hp))
            o_ps = [acc_ps.tile([128, 512], f32, tag="acc",
                                name=f"o_p{j}") for j in range(2)]
            if den_eng is None:
                den_eng = nc.sync
            for (kb, off, pt) in pts:
                for h2 in range(2):
                    nc.tensor.matmul(
                        o_ps[h2][0:65, off:512],
                        v_sb[:, kb, 2 * hp + h2, :],
                        pt[:, h2, off:512],
                        start=(kb == 0), stop=(kb == nkb - 1))
            csl = slice(c * 512, (c + 1) * 512)
            for h2 in range(2):
                u = u_pool.tile([65, 512], f32, tag="u")
                nc.vector.tensor_copy(u[:], o_ps[h2][0:65, :])
                den0 = nrm_pool.tile([1, 512], f32, tag="den0")
                den_eng.dma_start(den0[:], u[64:65, :])
                rcp0 = nrm_pool.tile([1, 512], f32, tag="rcp0")
                nc.vector.reciprocal_approx_fast(rcp0[:], den0[:])
                bc = nrm_pool.tile([64, 512], f32, tag="bc")
                nc.gpsimd.partition_broadcast(bc[:], rcp0[:])
                if h2 == 0:
                    nc.vector.tensor_tensor(attnT[hp][0:64, csl],
                                            u[0:64, :], bc[:], op=ALU.mult)
                else:
                    aTo = u_pool.tile([64, 512], bf16, tag="aTo")
                    nc.vector.tensor_tensor(aTo[:], u[0:64, :], bc[:],
                                            op=ALU.mult)
                    den_eng.dma_start(attnT[hp][64:128, csl], aTo[:])

        def proj_chunk(c, tail=False):
            csl = slice(c * 512, (c + 1) * 512)
            for dd in range(8):
                pp = acc_ps.tile([128, 512], f32, tag="acc", name="pp")
                for kc2 in range(2):
                    nc.tensor.matmul(
                        pp[:], wp_sb[:, kc2, dd * 128:(dd + 1) * 128],
                        attnT[kc2][:, csl],
                        start=(kc2 == 0), stop=(kc2 == 1))
                po = pj_sb.tile([128, 512], bf16, tag="po")
                if with_bias or (tail and dd % 2 == 0):
                    nc.scalar.activation(po[:], pp[:], AF.Identity,
                                         bias=bp_sb[:, dd:dd + 1])
                else:
                    nc.vector.tensor_copy(po[:], pp[:])
                eng = nc.gpsimd if dd % 2 == 0 else nc.sync
                if tail and dd >= 6:
                    eng = nc.scalar
                eng.dma_start(
                    outT_d.ap()[dd * 128:(dd + 1) * 128, csl], po[:])

        # ---------------- schedule ----------------
        # background PE tasks (fills/ropes/transposes) consumed between
        # score tiles: list of callables, each roughly 1-3.5us of PE.
        with nc.named_scope("s1"):
            # head-pair 0 q/k + rope; exp pipeline can start after this.
            q0raw = rope_pool.tile([128, S], bf16, tag="qraw", name="q0raw")
            k0raw = rope_pool.tile([128, S], bf16, tag="qraw", name="k0raw")
            fill_pair(0, (0, 1), q0raw)
            fill_pair(1, (0, 1), k0raw)
            rope_half(0, q0raw, 0)
            rope_half(1, k0raw, 0)
            fill_pair(0, (2, 3), q0raw)
            fill_pair(1, (2, 3), k0raw)
            rope_half(0, q0raw, 1)
            rope_half(1, k0raw, 1)

        q1raw = rope_pool.tile([128, S], bf16, tag="qraw", name="q1raw")
        k1raw = rope_pool.tile([128, S], bf16, tag="qraw", name="k1raw")
        bg = []
        bg.append(lambda: fill_pair(4, (0, 1), None))
        bg.append(lambda: fill_pair(4, (2, 3), None))
        bg.append(lambda: transposes(4))
        bg.append(lambda: fill_pair(5, (0, 1), None))
        bg.append(lambda: transposes(4))
        bg.append(lambda: fill_pair(5, (2, 3), None))
        bg.append(lambda: transposes(4))
        bg.append(lambda: fill_pair(2, (0, 1), q1raw))
        bg.append(lambda: transposes(4))
        bg.append(lambda: fill_pair(2, (2, 3), q1raw))
        bg.append(lambda: rope_half(2, q1raw, 0))
        bg.append(lambda: fill_pair(3, (0, 1), k1raw))
        bg.append(lambda: rope_half(2, q1raw, 1))
        bg.append(lambda: transposes(4))
        bg.append(lambda: fill_pair(3, (2, 3), k1raw))
        bg.append(lambda: rope_half(3, k1raw, 0))
        bg.append(lambda: transposes(6))
        bg.append(lambda: rope_half(3, k1raw, 1))
        bg.append(lambda: transposes(6))
        _bg = iter(bg)

        def bg_step(n=1):
            for _ in range(n):
                t = next(_bg, None)
                if t is None:
                    return
                t()

        # chunk order: hp0 ascending, hp1 descending; PV(prev) at the
        # midpoint of each chunk's scores; proj(c) once both hps done.
        chunks = [(0, 0), (1, 0), (2, 0), (3, 0),
                  (3, 1), (2, 1), (1, 1), (0, 1)]
        done = set()
        with nc.named_scope("attn"):
            prev = None
            for (c, hp) in chunks:
                nkb = 4 * c + 4
                pending_proj = None
                for kb in range(nkb):
                    if kb == nkb // 2 and prev is not None:
                        pv_pair(*prev)
                        done.add(prev)
                        if (prev[0], 1 - prev[1]) in done:
                            pending_proj = prev[0]
                        prev = None
                    scores_tile(c, hp, kb)
                    # one background task per ~2 score tiles early on,
                    # denser while hp0 is cheap to keep PE fed.
                    if hp == 0:
                        bg_step(1)
                if prev is not None:
                    pv_pair(*prev)
                    done.add(prev)
                    if (prev[0], 1 - prev[1]) in done:
                        pending_proj = prev[0]
                prev = (c, hp)
                if pending_proj is not None:
                    proj_chunk(pending_proj)
            bg_step(len(bg))
            pv_pair(0, 1, den_eng=nc.scalar)
            proj_chunk(0, tail=True)

    nc.finalize()
    return nc


def make_core_inputs(inputs, core, with_bias, _cache):
    """Host-side shard prep for one core. _cache is per-run (shared
    across the 4 cores of a batch)."""
    b, g = core // 4, core % 4

    if ("hT", b) not in _cache:
        hidden = np.asarray(inputs["hidden_states"], dtype=np.float32)
        _cache[("hT", b)] = np.ascontiguousarray(hidden[b].T).astype(BF)
    if ("trig", b) not in _cache:
        pos = np.asarray(inputs["position_ids"])
        inv_freq = (1.0 / (10000.0 **
                           (np.arange(0, 64, 2, dtype=np.float64) / 64.0)))
        freqsT = inv_freq[:, None] * pos[b].astype(np.float64)[None, :]
        embT = np.concatenate([freqsT, freqsT], axis=0)     # [64, S]
        cosp = np.cos(embT)
        sinp = np.sin(embT)
        sinp[:32, :] *= -1.0
        _cache[("trig", b)] = (np.tile(cosp, (2, 1)).astype(BF),
                               np.tile(sinp, (2, 1)).astype(BF))

    caw = np.asarray(inputs["c_attn_w"], dtype=np.float32)
    cab = np.asarray(inputs["c_attn_b"], dtype=np.float32)
    cpw = np.asarray(inputs["c_proj_w"], dtype=np.float32)
    cpb = np.asarray(inputs["c_proj_b"], dtype=np.float32)

    cs = slice(g * HD, (g + 1) * HD)
    qg = caw[:, cs]
    kg = caw[:, D + g * HD:D + (g + 1) * HD]
    vg = caw[:, 2 * D + g * HD:2 * D + (g + 1) * HD]
    # col blocks: [q01 | k01 | q23 | k23 | v0123]
    wqkv = np.concatenate(
        [qg[:, 0:128], kg[:, 0:128], qg[:, 128:256], kg[:, 128:256], vg],
        axis=1)

    bp = (cpb if g == 0 else np.zeros_like(cpb)).reshape(8, 128).T.copy()

    r = np.arange(128)
    mask01 = (r[None, :] >= r[:, None]).astype(BF)
    cosT, sinT = _cache[("trig", b)]

    out = {
        "hT": _cache[("hT", b)],
        "wqkv": np.ascontiguousarray(wqkv).astype(BF),
        "cosT": cosT,
        "sinT": sinT,
        "wp": np.ascontiguousarray(cpw[cs, :]).astype(BF),
        "bp": np.ascontiguousarray(bp.astype(np.float32)),
        "mask01": mask01,
        "ones64": np.ones((128, 64), BF),
        "ident": np.eye(128).astype(BF),
    }
    if with_bias:
        qb = cab[cs]
        kb = cab[D + g * HD:D + (g + 1) * HD]
        vb = cab[2 * D + g * HD:2 * D + (g + 1) * HD]
        bqkv = np.concatenate(
            [qb[0:128], kb[0:128], qb[128:256], kb[128:256], vb])[None, :]
        out["bqkv"] = bqkv.astype(BF)
        out["ones_row"] = np.ones((1, 512), BF)
    return out


_NC_CACHE = {}


def run(inputs, trace=False, **spmd_kwargs):
    """Shard, execute on 8 cores, unshard. Returns (output, BassKernelResults)."""
    with_bias = bool(np.any(np.asarray(inputs["c_attn_b"])) or
                     np.any(np.asarray(inputs["c_proj_b"])))
    if with_bias not in _NC_CACHE:
        _NC_CACHE[with_bias] = build_attention_nc(with_bias=with_bias,
                                                  num_devices=8)
    nc = _NC_CACHE[with_bias]
    prep_cache = {}
    in_maps = [make_core_inputs(inputs, c, with_bias, prep_cache)
               for c in range(8)]
    res = run_bass_kernel_spmd(nc, in_maps, core_ids=list(range(8)),
                               trace=trace, **spmd_kwargs)
    outs = []
    for b in range(2):
        acc = np.zeros((D, S), np.float32)
        for g in range(4):
            acc += res.results[b * 4 + g]["outT"].astype(np.float32)
        outs.append(acc.T)
    return np.stack(outs, axis=0), res


def kernel(**inputs) -> np.ndarray:
    out, _ = run(inputs, trace=False)
    return out


# revision 15
# speedup vs baseline: 1.1655x; 1.1655x over previous
"""TRN2 Bass kernel for GPT-style causal self-attention with RoPE (bf16).

Reference (B=2, S=2048, D=1024, H=16, dk=64):
  qkv = hidden @ c_attn_w + c_attn_b; rope(q), rope(k) via position_ids;
  out = softmax(causal(q k^T / 8)) v, merged heads, @ c_proj_w + c_proj_b.

Sharding across 8 NeuronCores: core c = 4*b + g handles batch b and head
group g (4 heads = 256 dims). Each core computes its full S x S attention
for its heads and a row-sliced c_proj partial; the host sums the 4
partials per batch.

v2 schedule: the Scalar engine's softmax exp (~76us busy) and the PE
(~115us busy) are the two near-critical engines; v1 started exp at
~55us. v2 computes head-pair 0's q/k first, ropes them in halves, and
starts the scores+exp pipeline at ~12us, interleaving the remaining
QKV (v, head-pair 1) and V transposes into the PE stream behind the
score tiles. Chunk order: hp0 ascending (0..3), hp1 descending (3..0)
so the tail ends on the smallest chunk. PV of the previous chunk is
emitted at the midpoint of the current chunk's scores; proj(c) fires
once both head-pairs of chunk c are normalized. One shared 3-buf PSUM
pool serves QKV fills, PV accumulators and proj so all phases coexist
with the 2x2-bank score pool and the transpose pool (8 banks total).

wqkv DRAM layout (host-side) is [q01|k01|q23|k23|v0123] so head-pair 0
needs one contiguous 256-col load per k-chunk. Input DMA is spread
over 5 engine queues (hT striped over sync/vector/tensor by k-chunk,
first column-half first; weights on scalar; trig/consts on gpsimd).

Output per core: outT [1024, 2048] bf16 partial; host sums per batch.
"""

from contextlib import ExitStack

import numpy as np
import ml_dtypes

import concourse.bacc as bacc
import concourse.tile as tile
import concourse.mybir as mybir
from concourse.bass_utils import run_bass_kernel_spmd

f32 = mybir.dt.float32
bf16 = mybir.dt.bfloat16
AF = mybir.ActivationFunctionType
ALU = mybir.AluOpType

S = 2048
D = 1024
HD = 256           # head dims per core (4 heads x 64)
SB = S // 128      # 16
KC = D // 128      # 8
NCH = S // 512     # 4
BF = ml_dtypes.bfloat16


def build_attention_nc(with_bias=False, num_devices=8):
    nc = bacc.Bacc("TRN2", target_bir_lowering=False, debug=False,
                   num_devices=num_devices)

    hT_d = nc.dram_tensor("hT", [D, S], bf16, kind="ExternalInput")
    wqkv_d = nc.dram_tensor("wqkv", [D, 768], bf16, kind="ExternalInput")
    cosT_d = nc.dram_tensor("cosT", [128, S], bf16, kind="ExternalInput")
    sinT_d = nc.dram_tensor("sinT", [128, S], bf16, kind="ExternalInput")
    wp_d = nc.dram_tensor("wp", [HD, D], bf16, kind="ExternalInput")
    bp_d = nc.dram_tensor("bp", [128, 8], f32, kind="ExternalInput")
    mask01_d = nc.dram_tensor("mask01", [128, 128], bf16, kind="ExternalInput")
    ones64_d = nc.dram_tensor("ones64", [128, 64], bf16, kind="ExternalInput")
    ident_d = nc.dram_tensor("ident", [128, 128], bf16, kind="ExternalInput")
    if with_bias:
        bqkv_d = nc.dram_tensor("bqkv", [1, 768], bf16, kind="ExternalInput")
        onesrow_d = nc.dram_tensor("ones_row", [1, 512], bf16,
                                   kind="ExternalInput")
    outT_d = nc.dram_tensor("outT", [D, S], bf16, kind="ExternalOutput")

    with tile.TileContext(nc) as tc, ExitStack() as top:
        const = top.enter_context(tc.tile_pool(name="const", bufs=1))
        ident = const.tile([128, 128], bf16, tag="ident")
        mask2 = const.tile([128, 2, 128], bf16, tag="mask2")
        bp_sb = const.tile([128, 8], f32, tag="bp")
        ones64 = const.tile([128, 64], bf16, tag="ones64")
        if with_bias:
            bqkv_sb = const.tile([1, 768], bf16, tag="bqkv")
            nc.sync.dma_start(bqkv_sb[:], bqkv_d.ap())
            ones_row = const.tile([1, 512], bf16, tag="ones_row")
            nc.sync.dma_start(ones_row[:], onesrow_d.ap())

        persist = top.enter_context(tc.tile_pool(name="persist", bufs=1))
        qT = [persist.tile([128, S], bf16, tag=f"qT{hp}", name=f"qT{hp}")
              for hp in range(2)]
        kT = [persist.tile([128, S], bf16, tag=f"kT{hp}", name=f"kT{hp}")
              for hp in range(2)]
        v_sb = persist.tile([128, SB, 4, 65], bf16, tag="v")
        wp_sb = persist.tile([128, 2, D], bf16, tag="wp")
        attnT = [persist.tile([128, S], bf16, tag=f"attnT{hp}",
                              name=f"attnT{hp}") for hp in range(2)]
        cosT = persist.tile([128, S], bf16, tag="cosT")
        sinT = persist.tile([128, S], bf16, tag="sinT")

        # ---------------- input DMA, spread over 5 queues ----------------
        # gpsimd: small consts + trig (needed by rope at ~7us)
        nc.gpsimd.dma_start(mask2[:, 0, :], mask01_d.ap())
        nc.gpsimd.dma_start(mask2[:, 1, :], mask01_d.ap())
        nc.gpsimd.dma_start(cosT[:], cosT_d.ap())
        nc.gpsimd.dma_start(sinT[:], sinT_d.ap())
        nc.gpsimd.dma_start(ident[:], ident_d.ap())
        nc.gpsimd.dma_start(ones64[:], ones64_d.ap())

        # Stage-A feed: fills consume (w[kc], hT[kc]-half0) in kc order.
        # sync: hT kc0-4 half0; scalar: w-hp0 then hT kc5-7 half0 (the
        # scalar queue is free again before exp starts at ~14us).
        w_pool = top.enter_context(tc.tile_pool(name="w", bufs=1))
        w_sb = [w_pool.tile([128, 768], bf16, tag=f"w{kc}", name=f"w{kc}")
                for kc in range(KC)]
        hT_pool = top.enter_context(tc.tile_pool(name="hT", bufs=1))
        hT_sb = [hT_pool.tile([128, S], bf16, tag=f"hT{kc}", name=f"hT{kc}")
                 for kc in range(KC)]
        for kc in range(KC):
            nc.scalar.dma_start(w_sb[kc][:, 0:256],
                                wqkv_d.ap()[kc * 128:(kc + 1) * 128, 0:256])
        for kc in range(5):
            nc.sync.dma_start(hT_sb[kc][:, 0:1024],
                              hT_d.ap()[kc * 128:(kc + 1) * 128, 0:1024])
        for kc in range(5, KC):
            nc.scalar.dma_start(hT_sb[kc][:, 0:1024],
                                hT_d.ap()[kc * 128:(kc + 1) * 128, 0:1024])
        for kc in range(5):
            nc.sync.dma_start(hT_sb[kc][:, 1024:S],
                              hT_d.ap()[kc * 128:(kc + 1) * 128, 1024:S])
        for kc in range(5, KC):
            nc.scalar.dma_start(hT_sb[kc][:, 1024:S],
                                hT_d.ap()[kc * 128:(kc + 1) * 128, 1024:S])
        # later-needed weights on sync, behind hT
        for kc in range(KC):
            nc.sync.dma_start(w_sb[kc][:, 512:768],
                              wqkv_d.ap()[kc * 128:(kc + 1) * 128, 512:768])
        for kc in range(KC):
            nc.sync.dma_start(w_sb[kc][:, 256:512],
                              wqkv_d.ap()[kc * 128:(kc + 1) * 128, 256:512])
        for kc2 in range(2):
            nc.sync.dma_start(wp_sb[:, kc2, :],
                              wp_d.ap()[kc2 * 128:(kc2 + 1) * 128, :])
        nc.gpsimd.dma_start(bp_sb[:], bp_d.ap())

        # ---------------- pools ----------------
        acc_ps = top.enter_context(
            tc.tile_pool(name="acc_ps", bufs=3, space="PSUM"))
        tr_ps = top.enter_context(
            tc.tile_pool(name="tr_ps", bufs=1, space="PSUM"))
        _lazy = {}

        def st_ps_tile():
            if "st" not in _lazy:
                _lazy["st"] = top.enter_context(
                    tc.tile_pool(name="st_ps", bufs=2, space="PSUM",
                                 side="right"))
                _lazy["n"] = 0
            _lazy["n"] += 1
            return _lazy["st"].tile([128, 2, 512], f32, tag="st_p",
                                    name=f"st_p{_lazy['n']}")

        pt_pool = top.enter_context(tc.tile_pool(name="pt", bufs=30))
        u_pool = top.enter_context(tc.tile_pool(name="u", bufs=3))
        nrm_pool = top.enter_context(tc.tile_pool(name="nrm", bufs=2))
        pj_sb = top.enter_context(tc.tile_pool(name="pj_sb", bufs=3))
        vT_pool = top.enter_context(tc.tile_pool(name="vT", bufs=1))
        rope_pool = top.enter_context(tc.tile_pool(name="rope", bufs=3))
        qsw_pool = top.enter_context(tc.tile_pool(name="qsw", bufs=2))
        qcos_pool = top.enter_context(tc.tile_pool(name="qcos", bufs=1))

        vT_sb = [vT_pool.tile([128, S], bf16, tag=f"vT{t}", name=f"vT{t}")
                 for t in range(2)]
        nc.scalar.copy(v_sb[:, :, :, 64],
                       ones64[:].rearrange("p (a b) -> p a b", a=SB))

        # ---------------- building blocks ----------------
        # w col blocks: 0=q01 1=k01 2=q23 3=k23 4,5=v
        def fill_pair(blk, spair, qraw, inter=None):
            sls = [slice(sb_ * 512, (sb_ + 1) * 512) for sb_ in spair]
            ps = [acc_ps.tile([128, 512], f32, tag="acc", name=f"qkv_p{j}")
                  for j, _ in enumerate(spair)]
            for kc in range(KC):
                for p, sl in zip(ps, sls):
                    nc.tensor.matmul(
                        p[:], w_sb[kc][:, blk * 128:(blk + 1) * 128],
                        hT_sb[kc][:, sl], start=(kc == 0),
                        stop=(kc == KC - 1 and not with_bias))
            if with_bias:
                for p in ps:
                    nc.tensor.matmul(
                        p[:], bqkv_sb[:, blk * 128:(blk + 1) * 128],
                        ones_row[:], start=False, stop=True)
            if inter is not None:
                inter()
            for p, sl in zip(ps, sls):
                if blk >= 4:
                    nc.vector.tensor_copy(vT_sb[blk - 4][:, sl], p[:])
                else:
                    nc.vector.tensor_copy(qraw[:, sl], p[:])

        # rope for one w-block over a 1024-col half.
        def rope_half(blk, qraw, half):
            dest = (qT if blk % 2 == 0 else kT)[blk // 2]
            cols = slice(half * 1024, (half + 1) * 1024)
            qsw = qsw_pool.tile([128, 1024], bf16, tag="qsw")
            for b4 in range(4):
                sp = (b4 * 32 + 32) % 64 + 64 * (b4 // 2)
                nc.gpsimd.dma_start(qsw[b4 * 32:b4 * 32 + 32, :],
                                    qraw[sp:sp + 32, cols])
            qcos = qcos_pool.tile([128, 1024], bf16, tag="qcos")
            nc.vector.tensor_tensor(qcos[:], qraw[:, cols], cosT[:, cols],
                                    op=ALU.mult)
            nc.vector.tensor_tensor(qsw[:], qsw[:], sinT[:, cols],
                                    op=ALU.mult)
            nc.vector.tensor_tensor(dest[:, cols], qcos[:], qsw[:],
                                    op=ALU.add)

        _tr = iter([(t, sb_) for t in range(2) for sb_ in range(SB)])

        def transposes(n):
            for _ in range(n):
                nxt = next(_tr, None)
                if nxt is None:
                    return
                t, sb_ = nxt
                tp = tr_ps.tile([128, 128], bf16, tag="tp")
                nc.tensor.matmul(
                    tp[:], vT_sb[t][:, sb_ * 128:(sb_ + 1) * 128],
                    ident[:], is_transpose=True, start=True, stop=True)
                nc.vector.tensor_copy(
                    v_sb[:, sb_, 2 * t:2 * t + 2, 0:64],
                    tp[:].rearrange("p (h d) -> p h d", h=2))

        pts_map = {}

        def scores_tile(c, hp, kb):
            q0 = max(512 * c, 128 * kb)
            off = q0 - 512 * c
            st_p = st_ps_tile()
            for h2 in range(2):
                nc.tensor.matmul(
                    st_p[:, h2, off:512],
                    kT[hp][h2 * 64:(h2 + 1) * 64, kb * 128:(kb + 1) * 128],
                    qT[hp][h2 * 64:(h2 + 1) * 64, q0:512 * (c + 1)],
                    start=True, stop=True, tile_position=(h2 * 64, 0))
            pt = pt_pool.tile([128, 2, 512], bf16, tag="pt")
            nc.scalar.activation(pt[:, :, off:512], st_p[:, :, off:512],
                                 AF.Exp, scale=0.125)
            if 128 * kb >= 512 * c:
                nc.vector.tensor_tensor(pt[:, :, off:off + 128],
                                        pt[:, :, off:off + 128],
                                        mask2[:], op=ALU.mult)
            pts_map.setdefault((c, hp), []).append((kb, off, pt))

        def pv_pair(c, hp, den_eng=None):
            nkb = 4 * c + 4
            pts = pts_map.pop((c, hp))
            o_ps = [acc_ps.tile([128, 512], f32, tag="acc",
                                name=f"o_p{j}") for j in range(2)]
            if den_eng is None:
                den_eng = nc.sync
            for (kb, off, pt) in pts:
                for h2 in range(2):
                    nc.tensor.matmul(
                        o_ps[h2][0:65, off:512],
                        v_sb[:, kb, 2 * hp + h2, :],
                        pt[:, h2, off:512],
                        start=(kb == 0), stop=(kb == nkb - 1))
            csl = slice(c * 512, (c + 1) * 512)
            # normalize: recip straight off u's denominator row (no DMA
            # hop); h2 chains interleaved so DVE/gpsimd overlap.
            us, dens, rcps = [], [], []
            for h2 in range(2):
                u = u_pool.tile([65, 512], f32, tag="u")
                nc.vector.tensor_copy(u[:], o_ps[h2][0:65, :])
                den0 = nrm_pool.tile([1, 512], f32, tag="den0")
                den_eng.dma_start(den0[:], u[64:65, :])
                us.append(u)
                dens.append(den0)
            for h2 in range(2):
                rcp0 = nrm_pool.tile([1, 512], f32, tag="rcp0")
                nc.vector.reciprocal_approx_fast(rcp0[:], dens[h2][:])
                rcps.append(rcp0)
            bcs = []
            for h2 in range(2):
                bc = nrm_pool.tile([64, 512], f32, tag="bc")
                nc.gpsimd.partition_broadcast(bc[:], rcps[h2][:])
                bcs.append(bc)
            nc.vector.tensor_tensor(attnT[hp][0:64, csl],
                                    us[0][0:64, :], bcs[0][:], op=ALU.mult)
            aTo = u_pool.tile([64, 512], bf16, tag="aTo")
            nc.vector.tensor_tensor(aTo[:], us[1][0:64, :], bcs[1][:],
                                    op=ALU.mult)
            den_eng.dma_start(attnT[hp][64:128, csl], aTo[:])

        def proj_chunk(c, tail=False):
            csl = slice(c * 512, (c + 1) * 512)
            for dd in range(8):
                pp = acc_ps.tile([128, 512], f32, tag="acc", name="pp")
                for kc2 in range(2):
                    nc.tensor.matmul(
                        pp[:], wp_sb[:, kc2, dd * 128:(dd + 1) * 128],
                        attnT[kc2][:, csl],
                        start=(kc2 == 0), stop=(kc2 == 1))
                po = pj_sb.tile([128, 512], bf16, tag="po")
                if with_bias or (tail and dd % 2 == 0):
                    nc.scalar.activation(po[:], pp[:], AF.Identity,
                                         bias=bp_sb[:, dd:dd + 1])
                else:
                    nc.vector.tensor_copy(po[:], pp[:])
                eng = nc.gpsimd if dd % 2 == 0 else nc.sync
                if tail and dd >= 6:
                    eng = nc.scalar
                eng.dma_start(
                    outT_d.ap()[dd * 128:(dd + 1) * 128, csl], po[:])

        # ---------------- schedule ----------------
        # background PE tasks (fills/ropes/transposes) consumed between
        # score tiles: list of callables, each roughly 1-3.5us of PE.
        with nc.named_scope("s1"):
            # head-pair 0 q/k + rope; exp pipeline can start after this.
            q0raw = rope_pool.tile([128, S], bf16, tag="qraw", name="q0raw")
            k0raw = rope_pool.tile([128, S], bf16, tag="qraw", name="k0raw")
            fill_pair(0, (0, 1), q0raw)
            fill_pair(1, (0, 1), k0raw)
            rope_half(0, q0raw, 0)
            rope_half(1, k0raw, 0)
            fill_pair(0, (2, 3), q0raw)
            fill_pair(1, (2, 3), k0raw)
            rope_half(0, q0raw, 1)
            rope_half(1, k0raw, 1)

        q1raw = rope_pool.tile([128, S], bf16, tag="qraw", name="q1raw")
        k1raw = rope_pool.tile([128, S], bf16, tag="qraw", name="k1raw")
        bg = []
        bg.append(lambda: fill_pair(4, (0, 1), None))
        bg.append(lambda: fill_pair(4, (2, 3), None))
        bg.append(lambda: transposes(4))
        bg.append(lambda: fill_pair(5, (0, 1), None))
        bg.append(lambda: transposes(4))
        bg.append(lambda: fill_pair(5, (2, 3), None))
        bg.append(lambda: transposes(4))
        bg.append(lambda: fill_pair(2, (0, 1), q1raw))
        bg.append(lambda: transposes(4))
        bg.append(lambda: fill_pair(2, (2, 3), q1raw))
        bg.append(lambda: rope_half(2, q1raw, 0))
        bg.append(lambda: fill_pair(3, (0, 1), k1raw))
        bg.append(lambda: rope_half(2, q1raw, 1))
        bg.append(lambda: transposes(4))
        bg.append(lambda: fill_pair(3, (2, 3), k1raw))
        bg.append(lambda: rope_half(3, k1raw, 0))
        bg.append(lambda: transposes(6))
        bg.append(lambda: rope_half(3, k1raw, 1))
        bg.append(lambda: transposes(6))
        _bg = iter(bg)

        def bg_step(n=1):
            for _ in range(n):
                t = next(_bg, None)
                if t is None:
                    return
                t()

        # chunk order: hp0 ascending, hp1 descending; PV(prev) at the
        # midpoint of each chunk's scores; proj(c) once both hps done.
        chunks = [(0, 0), (1, 0), (2, 0), (3, 0),
                  (3, 1), (2, 1), (0, 1), (1, 1)]
        done = set()
        with nc.named_scope("attn"):
            prev = None
            for (c, hp) in chunks:
                nkb = 4 * c + 4
                pending_proj = None
                for kb in range(nkb):
                    if kb == nkb // 2 and prev is not None:
                        pv_pair(*prev)
                        done.add(prev)
                        if (prev[0], 1 - prev[1]) in done:
                            pending_proj = prev[0]
                        prev = None
                    scores_tile(c, hp, kb)
                    # one background task per ~2 score tiles early on,
                    # denser while hp0 is cheap to keep PE fed.
                    if hp == 0:
                        bg_step(1)
                if prev is not None:
                    pv_pair(*prev)
                    done.add(prev)
                    if (prev[0], 1 - prev[1]) in done:
                        pending_proj = prev[0]
                prev = (c, hp)
                if pending_proj is not None:
                    proj_chunk(pending_proj)
            bg_step(len(bg))
            pv_pair(1, 1, den_eng=nc.scalar)
            proj_chunk(1, tail=True)

    nc.finalize()
    return nc


def make_core_inputs(inputs, core, with_bias, _cache):
    """Host-side shard prep for one core. _cache is per-run (shared
    across the 4 cores of a batch)."""
    b, g = core // 4, core % 4

    if ("hT", b) not in _cache:
        hidden = np.asarray(inputs["hidden_states"], dtype=np.float32)
        _cache[("hT", b)] = np.ascontiguousarray(hidden[b].T).astype(BF)
    if ("trig", b) not in _cache:
        pos = np.asarray(inputs["position_ids"])
        inv_freq = (1.0 / (10000.0 **
                           (np.arange(0, 64, 2, dtype=np.float64) / 64.0)))
        freqsT = inv_freq[:, None] * pos[b].astype(np.float64)[None, :]
        embT = np.concatenate([freqsT, freqsT], axis=0)     # [64, S]
        cosp = np.cos(embT)
        sinp = np.sin(embT)
        sinp[:32, :] *= -1.0
        _cache[("trig", b)] = (np.tile(cosp, (2, 1)).astype(BF),
                               np.tile(sinp, (2, 1)).astype(BF))

    caw = np.asarray(inputs["c_attn_w"], dtype=np.float32)
    cab = np.asarray(inputs["c_attn_b"], dtype=np.float32)
    cpw = np.asarray(inputs["c_proj_w"], dtype=np.float32)
    cpb = np.asarray(inputs["c_proj_b"], dtype=np.float32)

    cs = slice(g * HD, (g + 1) * HD)
    qg = caw[:, cs]
    kg = caw[:, D + g * HD:D + (g + 1) * HD]
    vg = caw[:, 2 * D + g * HD:2 * D + (g + 1) * HD]
    # col blocks: [q01 | k01 | q23 | k23 | v0123]
    wqkv = np.concatenate(
        [qg[:, 0:128], kg[:, 0:128], qg[:, 128:256], kg[:, 128:256], vg],
        axis=1)

    bp = (cpb if g == 0 else np.zeros_like(cpb)).reshape(8, 128).T.copy()

    r = np.arange(128)
    mask01 = (r[None, :] >= r[:, None]).astype(BF)
    cosT, sinT = _cache[("trig", b)]

    out = {
        "hT": _cache[("hT", b)],
        "wqkv": np.ascontiguousarray(wqkv).astype(BF),
        "cosT": cosT,
        "sinT": sinT,
        "wp": np.ascontiguousarray(cpw[cs, :]).astype(BF),
        "bp": np.ascontiguousarray(bp.astype(np.float32)),
        "mask01": mask01,
        "ones64": np.ones((128, 64), BF),
        "ident": np.eye(128).astype(BF),
    }
    if with_bias:
        qb = cab[cs]
        kb = cab[D + g * HD:D + (g + 1) * HD]
        vb = cab[2 * D + g * HD:2 * D + (g + 1) * HD]
        bqkv = np.concatenate(
            [qb[0:128], kb[0:128], qb[128:256], kb[128:256], vb])[None, :]
        out["bqkv"] = bqkv.astype(BF)
        out["ones_row"] = np.ones((1, 512), BF)
    return out


_NC_CACHE = {}


def run(inputs, trace=False, **spmd_kwargs):
    """Shard, execute on 8 cores, unshard. Returns (output, BassKernelResults)."""
    with_bias = bool(np.any(np.asarray(inputs["c_attn_b"])) or
                     np.any(np.asarray(inputs["c_proj_b"])))
    if with_bias not in _NC_CACHE:
        _NC_CACHE[with_bias] = build_attention_nc(with_bias=with_bias,
                                                  num_devices=8)
    nc = _NC_CACHE[with_bias]
    prep_cache = {}
    in_maps = [make_core_inputs(inputs, c, with_bias, prep_cache)
               for c in range(8)]
    res = run_bass_kernel_spmd(nc, in_maps, core_ids=list(range(8)),
                               trace=trace, **spmd_kwargs)
    outs = []
    for b in range(2):
        acc = np.zeros((D, S), np.float32)
        for g in range(4):
            acc += res.results[b * 4 + g]["outT"].astype(np.float32)
        outs.append(acc.T)
    return np.stack(outs, axis=0), res


def kernel(**inputs) -> np.ndarray:
    out, _ = run(inputs, trace=False)
    return out


# revision 21
# speedup vs baseline: 1.1701x; 1.0040x over previous
"""TRN2 Bass kernel for GPT-style causal self-attention with RoPE (bf16).

Reference (B=2, S=2048, D=1024, H=16, dk=64):
  qkv = hidden @ c_attn_w + c_attn_b; rope(q), rope(k) via position_ids;
  out = softmax(causal(q k^T / 8)) v, merged heads, @ c_proj_w + c_proj_b.

Sharding across 8 NeuronCores: core c = 4*b + g handles batch b and head
group g (4 heads = 256 dims). Each core computes its full S x S attention
for its heads and a row-sliced c_proj partial; the host sums the 4
partials per batch.

v4 clock-balanced weave: the two near-critical engines are the PE
(~115us of matmul) and ScalarE (~75us of softmax exp). The schedule
keeps both dense: after head-pair 0's q/k are computed and roped
(dense, ~14us of PE), every subsequent score tile is followed by
filler PE work popped from an ordered task queue (V/hp1 QKV fill
halves, V-transpose batches, PV kb-steps, proj dd-steps) until a
modeled PE clock catches a modeled Scalar clock, so exp starts at
~28us and never starves while the PE never bursts far ahead.
Score order: hp0 big-first (3,0),(2,0),(1,0),(0,0) during the
remaining QKV, then hp1 (3,1),(2,1),(0,1),(1,1). PV groups unlock
when the model says their exps are done; proj(c) unlocks ~1.5us
after both its normalize chains are emitted. The tail is a single
PV + normalize + proj.

Normalize chain per chunk-hp: PV's 65th row (ones column) holds the
softmax denominators; u-eviction -> den row DMA to partition 0 ->
fast reciprocal -> gpsimd partition-broadcast -> DVE multiply, with
the two head chains interleaved so DVE and gpsimd overlap.

wqkv DRAM layout (host-side) is [q01|k01|q23|k23|v0123] so head-pair 0
needs one contiguous 256-col load per k-chunk. hT is striped over the
sync (kc0-4) and scalar (kc5-7) DMA queues, first column-half first;
trig/consts on gpsimd; late weights on sync behind hT.

Output per core: outT [1024, 2048] bf16 partial; host sums per batch.
"""

from contextlib import ExitStack

import numpy as np
import ml_dtypes

import concourse.bacc as bacc
import concourse.tile as tile
import concourse.mybir as mybir
from concourse.bass_utils import run_bass_kernel_spmd

f32 = mybir.dt.float32
bf16 = mybir.dt.bfloat16
AF = mybir.ActivationFunctionType
ALU = mybir.AluOpType

S = 2048
D = 1024
HD = 256           # head dims per core (4 heads x 64)
SB = S // 128      # 16
KC = D // 128      # 8
NCH = S // 512     # 4
BF = ml_dtypes.bfloat16

PE_NS = 0.417e-3   # us per PE cycle at full clock
EXP_NS = 1.1e-3    # us per element-column of Scalar exp


def build_attention_nc(with_bias=False, num_devices=8):
    nc = bacc.Bacc("TRN2", target_bir_lowering=False, debug=False,
                   num_devices=num_devices)

    hT_d = nc.dram_tensor("hT", [D, S], bf16, kind="ExternalInput")
    wqkv_d = nc.dram_tensor("wqkv", [D, 768], bf16, kind="ExternalInput")
    cosT_d = nc.dram_tensor("cosT", [128, S], bf16, kind="ExternalInput")
    sinT_d = nc.dram_tensor("sinT", [128, S], bf16, kind="ExternalInput")
    wp_d = nc.dram_tensor("wp", [HD, D], bf16, kind="ExternalInput")
    bp_d = nc.dram_tensor("bp", [128, 8], f32, kind="ExternalInput")
    mask01_d = nc.dram_tensor("mask01", [128, 128], bf16, kind="ExternalInput")
    ones64_d = nc.dram_tensor("ones64", [128, 64], bf16, kind="ExternalInput")
    ident_d = nc.dram_tensor("ident", [128, 128], bf16, kind="ExternalInput")
    if with_bias:
        bqkv_d = nc.dram_tensor("bqkv", [1, 768], bf16, kind="ExternalInput")
        onesrow_d = nc.dram_tensor("ones_row", [1, 512], bf16,
                                   kind="ExternalInput")
    outT_d = nc.dram_tensor("outT", [D, S], bf16, kind="ExternalOutput")

    with tile.TileContext(nc) as tc, ExitStack() as top:
        const = top.enter_context(tc.tile_pool(name="const", bufs=1))
        ident = const.tile([128, 128], bf16, tag="ident")
        mask2 = const.tile([128, 2, 128], bf16, tag="mask2")
        bp_sb = const.tile([128, 8], f32, tag="bp")
        ones64 = const.tile([128, 64], bf16, tag="ones64")
        if with_bias:
            bqkv_sb = const.tile([1, 768], bf16, tag="bqkv")
            nc.sync.dma_start(bqkv_sb[:], bqkv_d.ap())
            ones_row = const.tile([1, 512], bf16, tag="ones_row")
            nc.sync.dma_start(ones_row[:], onesrow_d.ap())

        persist = top.enter_context(tc.tile_pool(name="persist", bufs=1))
        qT = [persist.tile([128, S], bf16, tag=f"qT{hp}", name=f"qT{hp}")
              for hp in range(2)]
        kT = [persist.tile([128, S], bf16, tag=f"kT{hp}", name=f"kT{hp}")
              for hp in range(2)]
        v_sb = persist.tile([128, SB, 4, 65], bf16, tag="v")
        wp_sb = persist.tile([128, 2, D], bf16, tag="wp")
        attnT = [persist.tile([128, S], bf16, tag=f"attnT{hp}",
                              name=f"attnT{hp}") for hp in range(2)]
        cosT = persist.tile([128, S], bf16, tag="cosT")
        sinT = persist.tile([128, S], bf16, tag="sinT")

        # ---------------- input DMA ----------------
        nc.gpsimd.dma_start(mask2[:, 0, :], mask01_d.ap())
        nc.gpsimd.dma_start(mask2[:, 1, :], mask01_d.ap())
        nc.gpsimd.dma_start(cosT[:], cosT_d.ap())
        nc.gpsimd.dma_start(sinT[:], sinT_d.ap())
        nc.gpsimd.dma_start(ident[:], ident_d.ap())
        nc.gpsimd.dma_start(ones64[:], ones64_d.ap())

        w_pool = top.enter_context(tc.tile_pool(name="w", bufs=1))
        w_sb = [w_pool.tile([128, 768], bf16, tag=f"w{kc}", name=f"w{kc}")
                for kc in range(KC)]
        hT_pool = top.enter_context(tc.tile_pool(name="hT", bufs=1))
        hT_sb = [hT_pool.tile([128, S], bf16, tag=f"hT{kc}", name=f"hT{kc}")
                 for kc in range(KC)]
        for kc in range(KC):
            nc.scalar.dma_start(w_sb[kc][:, 0:256],
                                wqkv_d.ap()[kc * 128:(kc + 1) * 128, 0:256])
        for kc in range(5):
            nc.sync.dma_start(hT_sb[kc][:, 0:1024],
                              hT_d.ap()[kc * 128:(kc + 1) * 128, 0:1024])
        for kc in range(5, KC):
            nc.scalar.dma_start(hT_sb[kc][:, 0:1024],
                                hT_d.ap()[kc * 128:(kc + 1) * 128, 0:1024])
        for kc in range(5):
            nc.sync.dma_start(hT_sb[kc][:, 1024:S],
                              hT_d.ap()[kc * 128:(kc + 1) * 128, 1024:S])
        for kc in range(5, KC):
            nc.scalar.dma_start(hT_sb[kc][:, 1024:S],
                                hT_d.ap()[kc * 128:(kc + 1) * 128, 1024:S])
        for kc in range(KC):
            nc.sync.dma_start(w_sb[kc][:, 512:768],
                              wqkv_d.ap()[kc * 128:(kc + 1) * 128, 512:768])
        for kc in range(KC):
            nc.sync.dma_start(w_sb[kc][:, 256:512],
                              wqkv_d.ap()[kc * 128:(kc + 1) * 128, 256:512])
        for kc2 in range(2):
            nc.sync.dma_start(wp_sb[:, kc2, :],
                              wp_d.ap()[kc2 * 128:(kc2 + 1) * 128, :])
        nc.gpsimd.dma_start(bp_sb[:], bp_d.ap())

        # ---------------- pools ----------------
        acc_ps = top.enter_context(
            tc.tile_pool(name="acc_ps", bufs=3, space="PSUM"))
        tr_ps = top.enter_context(
            tc.tile_pool(name="tr_ps", bufs=1, space="PSUM"))
        _lazy = {}

        def st_ps_tile():
            if "st" not in _lazy:
                _lazy["st"] = top.enter_context(
                    tc.tile_pool(name="st_ps", bufs=2, space="PSUM",
                                 side="right"))
                _lazy["n"] = 0
            _lazy["n"] += 1
            return _lazy["st"].tile([128, 2, 512], f32, tag="st_p",
                                    name=f"st_p{_lazy['n']}")

        pt_pool = top.enter_context(tc.tile_pool(name="pt", bufs=32))
        u_pool = top.enter_context(tc.tile_pool(name="u", bufs=3))
        nrm_pool = top.enter_context(tc.tile_pool(name="nrm", bufs=2))
        pj_sb = top.enter_context(tc.tile_pool(name="pj_sb", bufs=3))
        vT_pool = top.enter_context(tc.tile_pool(name="vT", bufs=1))
        rope_pool = top.enter_context(tc.tile_pool(name="rope", bufs=2))
        qsw_pool = top.enter_context(tc.tile_pool(name="qsw", bufs=1))
        qcos_pool = top.enter_context(tc.tile_pool(name="qcos", bufs=1))

        vT_sb = [vT_pool.tile([128, S], bf16, tag=f"vT{t}", name=f"vT{t}")
                 for t in range(2)]
        nc.scalar.copy(v_sb[:, :, :, 64],
                       ones64[:].rearrange("p (a b) -> p a b", a=SB))

        # ---------------- building blocks ----------------
        # w col blocks: 0=q01 1=k01 2=q23 3=k23 4,5=v
        def fill_pair(blk, spair, qraw):
            sls = [slice(sb_ * 512, (sb_ + 1) * 512) for sb_ in spair]
            ps = [acc_ps.tile([128, 512], f32, tag="acc", name=f"qkv_p{j}")
                  for j, _ in enumerate(spair)]

            def half(h):
                for kc in range(4 * h, 4 * h + 4):
                    for p, sl in zip(ps, sls):
                        nc.tensor.matmul(
                            p[:], w_sb[kc][:, blk * 128:(blk + 1) * 128],
                            hT_sb[kc][:, sl], start=(kc == 0),
                            stop=(kc == KC - 1 and not with_bias))
                if h == 1:
                    if with_bias:
                        for p in ps:
                            nc.tensor.matmul(
                                p[:], bqkv_sb[:, blk * 128:(blk + 1) * 128],
                                ones_row[:], start=False, stop=True)
                    for p, sl in zip(ps, sls):
                        if blk >= 4:
                            nc.vector.tensor_copy(vT_sb[blk - 4][:, sl], p[:])
                        else:
                            nc.vector.tensor_copy(qraw[:, sl], p[:])
            return half

        def rope_chunk(blk, qraw):
            dest = (qT if blk % 2 == 0 else kT)[blk // 2]
            qsw = qsw_pool.tile([128, S], bf16, tag="qsw")
            for b4 in range(4):
                sp = (b4 * 32 + 32) % 64 + 64 * (b4 // 2)
                nc.gpsimd.dma_start(qsw[b4 * 32:b4 * 32 + 32, :],
                                    qraw[sp:sp + 32, :])
            qcos = qcos_pool.tile([128, S], bf16, tag="qcos")
            nc.vector.tensor_tensor(qcos[:], qraw[:], cosT[:], op=ALU.mult)
            nc.vector.tensor_tensor(qsw[:], qsw[:], sinT[:], op=ALU.mult)
            nc.vector.tensor_tensor(dest[:], qcos[:], qsw[:], op=ALU.add)

        _tr = iter([(t, sb_) for t in range(2) for sb_ in range(SB)])

        def transposes(n):
            for _ in range(n):
                nxt = next(_tr, None)
                if nxt is None:
                    return
                t, sb_ = nxt
                tp = tr_ps.tile([128, 128], bf16, tag="tp")
                nc.tensor.matmul(
                    tp[:], vT_sb[t][:, sb_ * 128:(sb_ + 1) * 128],
                    ident[:], is_transpose=True, start=True, stop=True)
                nc.vector.tensor_copy(
                    v_sb[:, sb_, 2 * t:2 * t + 2, 0:64],
                    tp[:].rearrange("p (h d) -> p h d", h=2))

        pts_map = {}

        def scores_tile(c, hp, kb):
            q0 = max(512 * c, 128 * kb)
            off = q0 - 512 * c
            st_p = st_ps_tile()
            for h2 in range(2):
                nc.tensor.matmul(
                    st_p[:, h2, off:512],
                    kT[hp][h2 * 64:(h2 + 1) * 64, kb * 128:(kb + 1) * 128],
                    qT[hp][h2 * 64:(h2 + 1) * 64, q0:512 * (c + 1)],
                    start=True, stop=True, tile_position=(h2 * 64, 0))
            pt = pt_pool.tile([128, 2, 512], bf16, tag="pt")
            nc.scalar.activation(pt[:, :, off:512], st_p[:, :, off:512],
                                 AF.Exp, scale=0.125)
            if 128 * kb >= 512 * c:
                nc.vector.tensor_tensor(pt[:, :, off:off + 128],
                                        pt[:, :, off:off + 128],
                                        mask2[:], op=ALU.mult)
            pts_map.setdefault((c, hp), []).append((kb, off, pt))

        def pv_norm(c, hp, o_ps, den_eng):
            csl = slice(c * 512, (c + 1) * 512)
            us, dens, rcps = [], [], []
            for h2 in range(2):
                u = u_pool.tile([65, 512], f32, tag="u")
                nc.vector.tensor_copy(u[:], o_ps[h2][0:65, :])
                den0 = nrm_pool.tile([1, 512], f32, tag="den0")
                den_eng.dma_start(den0[:], u[64:65, :])
                us.append(u)
                dens.append(den0)
            for h2 in range(2):
                rcp0 = nrm_pool.tile([1, 512], f32, tag="rcp0")
                nc.vector.reciprocal_approx_fast(rcp0[:], dens[h2][:])
                rcps.append(rcp0)
            bcs = []
            for h2 in range(2):
                bc = nrm_pool.tile([64, 512], f32, tag="bc")
                nc.gpsimd.partition_broadcast(bc[:], rcps[h2][:])
                bcs.append(bc)
            nc.vector.tensor_tensor(attnT[hp][0:64, csl],
                                    us[0][0:64, :], bcs[0][:], op=ALU.mult)
            aTo = u_pool.tile([64, 512], bf16, tag="aTo")
            nc.vector.tensor_tensor(aTo[:], us[1][0:64, :], bcs[1][:],
                                    op=ALU.mult)
            den_eng.dma_start(attnT[hp][64:128, csl], aTo[:])

        # ---------------- clock-balanced weave ----------------
        clk = {"pe": 0.0, "sc": 0.0}
        exp_end = {}
        norm_pe = {}

        def sc_emit(c, hp, kb):
            off = max(512 * c, 128 * kb) - 512 * c
            scores_tile(c, hp, kb)
            clk["pe"] += 2 * (512 - off) * PE_NS
            clk["sc"] = max(clk["sc"], clk["pe"]) \
                + 2 * (512 - off) * EXP_NS + 0.25

        filler = []

        def add_task(emit, cost, ready=None):
            filler.append([ready, emit, cost])

        def add_fill(blk, spair, qraw=None):
            half = fill_pair(blk, spair, qraw)
            add_task(lambda: half(0), 4 * 512 * PE_NS)
            add_task(lambda: half(1), 4 * 512 * PE_NS)

        def add_pv(c, hp, den_eng=None):
            nkb = 4 * c + 4
            state = {}

            def first():
                state["o"] = [acc_ps.tile([128, 512], f32, tag="acc",
                                          name=f"o_p{j}") for j in range(2)]

            def step(i):
                if i == 0:
                    first()
                kb, off, pt = pts_map[(c, hp)][i]
                for h2 in range(2):
                    nc.tensor.matmul(
                        state["o"][h2][0:65, off:512],
                        v_sb[:, kb, 2 * hp + h2, :],
                        pt[:, h2, off:512],
                        start=(kb == 0), stop=(kb == nkb - 1))
                if i == nkb - 1:
                    pv_norm(c, hp, state["o"], den_eng or nc.sync)
                    del pts_map[(c, hp)]
                    norm_pe[(c, hp)] = clk["pe"]

            def ready():
                return clk["sc"] >= exp_end.get((c, hp), 1e9)
            for i in range(nkb):
                off = pts_lookup_off(c, i)
                add_task(lambda i=i: step(i), 2 * (512 - off) * PE_NS,
                         ready if i == 0 else None)

        def pts_lookup_off(c, i):
            # kb index i of chunk c has offset max(0, 128*(i) - 0) pattern:
            # kb runs 0..4c+3; off = max(0, 128*kb - 512*c)
            return max(0, 128 * i - 512 * c)

        def add_proj(c, tail=False):
            def ready():
                return ((c, 0) in norm_pe and (c, 1) in norm_pe
                        and clk["pe"] >= max(norm_pe[(c, 0)],
                                             norm_pe[(c, 1)]) + 1.5)

            def step(dd):
                csl = slice(c * 512, (c + 1) * 512)
                pp = acc_ps.tile([128, 512], f32, tag="acc", name="pp")
                for kc2 in range(2):
                    nc.tensor.matmul(
                        pp[:], wp_sb[:, kc2, dd * 128:(dd + 1) * 128],
                        attnT[kc2][:, csl],
                        start=(kc2 == 0), stop=(kc2 == 1))
                po = pj_sb.tile([128, 512], bf16, tag="po")
                if with_bias or (tail and dd % 2 == 0):
                    nc.scalar.activation(po[:], pp[:], AF.Identity,
                                         bias=bp_sb[:, dd:dd + 1])
                else:
                    nc.vector.tensor_copy(po[:], pp[:])
                eng = nc.gpsimd if dd % 2 == 0 else nc.sync
                if tail and dd >= 6:
                    eng = nc.scalar
                eng.dma_start(
                    outT_d.ap()[dd * 128:(dd + 1) * 128, csl], po[:])
            for dd in range(8):
                add_task(lambda dd=dd: step(dd), 2 * 512 * PE_NS,
                         ready if dd == 0 else None)

        fidx = [0]

        def weave(force=False):
            while fidx[0] < len(filler):
                ready, emit, cost = filler[fidx[0]]
                if not force and clk["pe"] >= clk["sc"] + 0.1:
                    return
                if not force and ready is not None and not ready():
                    return
                emit()
                clk["pe"] += cost
                fidx[0] += 1

        def weave_until(idx):
            # hard flush: guarantees emission-order prerequisites (rope
            # before hp1 scores, PV pops before the pt pool wraps).
            while fidx[0] < idx:
                ready, emit, cost = filler[fidx[0]]
                emit()
                clk["pe"] += cost
                fidx[0] += 1

        # ---------------- schedule ----------------
        with nc.named_scope("s1"):
            q0raw = rope_pool.tile([128, S], bf16, tag="qraw", name="q0raw")
            k0raw = rope_pool.tile([128, S], bf16, tag="qraw", name="k0raw")
            for blk, raw in ((0, q0raw), (1, k0raw)):
                for spair in ((0, 1), (2, 3)):
                    h = fill_pair(blk, spair, raw)
                    h(0)
                    h(1)
            rope_chunk(0, q0raw)
            rope_chunk(1, k0raw)
            clk["pe"] += 4 * 8192 * PE_NS + 12.0   # fills + start lag

        q1raw = rope_pool.tile([128, S], bf16, tag="qraw", name="q1raw")
        k1raw = rope_pool.tile([128, S], bf16, tag="qraw", name="k1raw")

        add_fill(4, (0, 1))
        add_fill(4, (2, 3))
        for _ in range(4):
            add_task(lambda: transposes(4), 4 * 128 * PE_NS)
        add_fill(5, (0, 1))
        add_fill(5, (2, 3))
        for _ in range(4):
            add_task(lambda: transposes(4), 4 * 128 * PE_NS)
        add_fill(2, (0, 1), q1raw)
        add_fill(2, (2, 3), q1raw)
        add_task(lambda: rope_chunk(2, q1raw), 0.1)
        add_fill(3, (0, 1), k1raw)
        add_fill(3, (2, 3), k1raw)
        add_task(lambda: rope_chunk(3, k1raw), 0.1)
        add_pv(3, 0)
        pv30_end = len(filler)
        add_pv(2, 0)
        pv20_end = len(filler)
        add_pv(1, 0)
        add_pv(0, 0)
        pv00_end = len(filler)
        add_pv(3, 1)
        pv31_end = len(filler)
        add_proj(3)
        add_pv(2, 1)
        add_proj(2)
        add_pv(0, 1)
        add_proj(0)
        add_pv(1, 1, den_eng=nc.scalar)
        add_proj(1, tail=True)

        # flush markers keep the pt pool from wrapping onto live tiles
        # and guarantee rope(3) precedes head-pair-1 scores.
        flush_before = {(1, 0): pv30_end, (3, 1): pv20_end,
                        (2, 1): pv00_end, (1, 1): pv31_end}
        sc_order = [(3, 0), (2, 0), (1, 0), (0, 0),
                    (3, 1), (2, 1), (0, 1), (1, 1)]
        with nc.named_scope("attn"):
            for (c, hp) in sc_order:
                if (c, hp) in flush_before:
                    weave_until(flush_before[(c, hp)])
                for kb in range(4 * c + 4):
                    sc_emit(c, hp, kb)
                    weave()
                exp_end[(c, hp)] = clk["sc"]
            weave(force=True)

    nc.finalize()
    return nc


def make_core_inputs(inputs, core, with_bias, _cache):
    """Host-side shard prep for one core. _cache is per-run (shared
    across the 4 cores of a batch)."""
    b, g = core // 4, core % 4

    if ("hT", b) not in _cache:
        hidden = np.asarray(inputs["hidden_states"], dtype=np.float32)
        _cache[("hT", b)] = np.ascontiguousarray(hidden[b].T).astype(BF)
    if ("trig", b) not in _cache:
        pos = np.asarray(inputs["position_ids"])
        inv_freq = (1.0 / (10000.0 **
                           (np.arange(0, 64, 2, dtype=np.float64) / 64.0)))
        freqsT = inv_freq[:, None] * pos[b].astype(np.float64)[None, :]
        embT = np.concatenate([freqsT, freqsT], axis=0)     # [64, S]
        cosp = np.cos(embT)
        sinp = np.sin(embT)
        sinp[:32, :] *= -1.0
        _cache[("trig", b)] = (np.tile(cosp, (2, 1)).astype(BF),
                               np.tile(sinp, (2, 1)).astype(BF))

    caw = np.asarray(inputs["c_attn_w"], dtype=np.float32)
    cab = np.asarray(inputs["c_attn_b"], dtype=np.float32)
    cpw = np.asarray(inputs["c_proj_w"], dtype=np.float32)
    cpb = np.asarray(inputs["c_proj_b"], dtype=np.float32)

    cs = slice(g * HD, (g + 1) * HD)
    qg = caw[:, cs]
    kg = caw[:, D + g * HD:D + (g + 1) * HD]
    vg = caw[:, 2 * D + g * HD:2 * D + (g + 1) * HD]
    # col blocks: [q01 | k01 | q23 | k23 | v0123]
    wqkv = np.concatenate(
        [qg[:, 0:128], kg[:, 0:128], qg[:, 128:256], kg[:, 128:256], vg],
        axis=1)

    bp = (cpb if g == 0 else np.zeros_like(cpb)).reshape(8, 128).T.copy()

    r = np.arange(128)
    mask01 = (r[None, :] >= r[:, None]).astype(BF)
    cosT, sinT = _cache[("trig", b)]

    out = {
        "hT": _cache[("hT", b)],
        "wqkv": np.ascontiguousarray(wqkv).astype(BF),
        "cosT": cosT,
        "sinT": sinT,
        "wp": np.ascontiguousarray(cpw[cs, :]).astype(BF),
        "bp": np.ascontiguousarray(bp.astype(np.float32)),
        "mask01": mask01,
        "ones64": np.ones((128, 64), BF),
        "ident": np.eye(128).astype(BF),
    }
    if with_bias:
        qb = cab[cs]
        kb = cab[D + g * HD:D + (g + 1) * HD]
        vb = cab[2 * D + g * HD:2 * D + (g + 1) * HD]
        bqkv = np.concatenate(
            [qb[0:128], kb[0:128], qb[128:256], kb[128:256], vb])[None, :]
        out["bqkv"] = bqkv.astype(BF)
        out["ones_row"] = np.ones((1, 512), BF)
    return out


_NC_CACHE = {}


def run(inputs, trace=False, **spmd_kwargs):
    """Shard, execute on 8 cores, unshard. Returns (output, BassKernelResults)."""
    with_bias = bool(np.any(np.asarray(inputs["c_attn_b"])) or
                     np.any(np.asarray(inputs["c_proj_b"])))
    if with_bias not in _NC_CACHE:
        _NC_CACHE[with_bias] = build_attention_nc(with_bias=with_bias,
                                                  num_devices=8)
    nc = _NC_CACHE[with_bias]
    prep_cache = {}
    in_maps = [make_core_inputs(inputs, c, with_bias, prep_cache)
               for c in range(8)]
    res = run_bass_kernel_spmd(nc, in_maps, core_ids=list(range(8)),
                               trace=trace, **spmd_kwargs)
    outs = []
    for b in range(2):
        acc = np.zeros((D, S), np.float32)
        for g in range(4):
            acc += res.results[b * 4 + g]["outT"].astype(np.float32)
        outs.append(acc.T)
    return np.stack(outs, axis=0), res


def kernel(**inputs) -> np.ndarray:
    out, _ = run(inputs, trace=False)
    return out
